# revision 1
# baseline (speedup 1.0000x reference)
"""Trainium2 Bass kernel for nn_CEClassifier: EDM Euler sampler (18 steps,
3x3 conv denoiser surrogate) + classifier head + pairwise logsumexp.

Strategy (8 NeuronCores, data-parallel over the n_ces*B=128 sampler rows):
  - Core k handles batch rows {8k..8k+8} U {64+8k..64+8k+8}  (both CE copies
    of image rows 8k..8k+8, so the final logsumexp over CEs is core-local).
  - The whole per-step update is folded (host-side) into
        x_{s+1} = conv(x_s, Weff_s) + noise'_s
    where Weff_s = B_s*c_in_s*W_net + A_s*I and
    noise'_s = S_s*eps_s + C_s*mu + B_s*b_net  (all scalars known at build).
  - The 3x3 conv runs on the TensorEngine as width-Toeplitz matmuls:
    K=(w,c)=102 partitions (32 interior w + halo w + pad w), M=(w_out,c)=96,
    3 accumulating matmuls (one per dy row-shift, realized as free-dim AP
    offsets into the H-padded state) + one identity matmul that injects
    noise' into PSUM. State x is SBUF-resident fp16, two width-chunks,
    ping-pong buffered across steps; PSUM then holds the complete x_{s+1},
    so the per-step epilogue is a DVE copy (interior) + ACT copy (halo
    column into the other chunk).  Chunk 0's output order is rotated
    (w31 first) so both chunks' halo sources sit at psum partitions [0:3).
  - Classifier: x staged to DRAM, re-read feature-major; W_cls streamed as
    fp16 [24 x 128 x 4 x 1000] tiles; 16x1000 logits accumulated in PSUM;
    exp -> pair-sum (tiny matmul) -> ln(0.5*x) gives logsumexp-log(2).
"""

import os
import numpy as np

# ---- problem constants (hardcoded per contest contract) ----
NUM_STEPS = 18
SIGMA_MIN = 0.002
SIGMA_MAX = 80.0
RHO = 7.0
CE_SIGMA = 0.2
SIGMA_DATA = 0.5
N_CES = 2
B, C, H, W = 64, 3, 64, 64
NUM_CLASSES = 1000
NCORES = 8
BPC = B // NCORES        # image rows per core (8)
BS = N_CES * BPC         # sampler rows per core (16)
HW_PAD = H + 2           # 66
KP = 102                 # conv K partitions: 96 interior + 3 halo + 3 pad
MP = 96                  # conv M partitions (32 w_out x 3 ch)
NKC = 96                 # classifier K chunks (12288/128)
W2GRP = 4                # K-chunks per W2 DMA group
NW2G = NKC // W2GRP      # 24 groups
W2BUFS = 16              # W2 group tiles resident

F16 = np.float16
F32 = np.float32


def _wmap(q):
    """partition index p in [0,96) -> global w value for chunk q."""
    p = np.arange(MP)
    if q == 0:
        return (p // 3 + 31) % 32          # rotated: w31, w0, w1, ..., w30
    return 32 + p // 3                     # standard: w32, ..., w63


def _t_steps():
    i = np.arange(NUM_STEPS, dtype=np.float64)
    ts = (SIGMA_MAX ** (1.0 / RHO) + i / (NUM_STEPS - 1) *
          (SIGMA_MIN ** (1.0 / RHO) - SIGMA_MAX ** (1.0 / RHO))) ** RHO
    return np.concatenate([ts, np.zeros(1)]).astype(np.float32)


def _step_coeffs():
    ts = _t_steps().astype(np.float64)
    out = []
    for s in range(NUM_STEPS):
        t, tn = ts[s], ts[s + 1]
        s2 = t * t
        denom = s2 + SIGMA_DATA ** 2
        c_skip = SIGMA_DATA ** 2 / denom
        c_out = t * SIGMA_DATA / np.sqrt(denom)
        c_in = 1.0 / np.sqrt(denom)
        dt2 = 2.0 * (t - tn)
        A = 1.0 + dt2 * ((c_skip - 1.0) / t - t / (CE_SIGMA ** 2 + s2))
        Bs = dt2 * c_out / t
        Cs = dt2 * t / (CE_SIGMA ** 2 + s2)
        Ss = np.sqrt(2.0 * t * (t - tn))
        out.append((A, Bs * c_in, Bs, Cs, Ss))
    return out, ts


def _build_toeplitz(W_net):
    """wts[102, 18*3*2, 96] fp16: column block j=((s*3+dy)*2+q)."""
    coeffs, _ = _step_coeffs()
    I3 = np.zeros((C, C, 3, 3), np.float64)
    for c in range(C):
        I3[c, c, 1, 1] = 1.0
    wts = np.zeros((KP, NUM_STEPS * 3 * 2, MP), np.float64)
    for s in range(NUM_STEPS):
        A, Bc, Bs, Cs, Ss = coeffs[s]
        Weff = Bc * W_net.astype(np.float64) + A * I3  # [o, c, dy, dx]
        for q in range(2):
            wrow = np.zeros(KP, np.int64)
            crow = np.zeros(KP, np.int64)
            valid = np.ones(KP, bool)
            wrow[:MP] = _wmap(q)
            crow[:MP] = np.arange(MP) % 3
            wrow[96:99] = 32 if q == 0 else 31   # active halo rows
            crow[96:99] = np.arange(3)
            valid[99:] = False                   # pad rows: zero weights
            wcol = _wmap(q)                      # psum/M order, same rotation
            for dy in range(3):
                col = (s * 3 + dy) * 2 + q
                for p in range(KP):
                    if not valid[p]:
                        continue
                    for m in range(MP):
                        dglob = wrow[p] - wcol[m]
                        if -1 <= dglob <= 1:
                            wts[p, col, m] = Weff[m % 3, crow[p], dy, dglob + 1]
    return wts.astype(F16)


def _host_prep(core, x, latents, noise, W_net, b_net, W_cls, b_cls, shared):
    """Build the per-core input arrays (partition-major device layouts)."""
    coeffs, ts = _step_coeffs()
    rows = np.concatenate([np.arange(BPC * core, BPC * core + BPC),
                           64 + np.arange(BPC * core, BPC * core + BPC)])
    mu = 2.0 * x[rows % 64].astype(np.float64) - 1.0       # [16, C, H, W]
    x0 = (latents[rows].astype(np.float64) * ts[0])        # [16, C, H, W]

    wm = [_wmap(0), _wmap(1)]
    cm = np.arange(MP) % 3

    # x_init [2, 102, 66, 16]
    xi = np.zeros((2, KP, HW_PAD, BS), F32)
    x0t = x0.transpose(3, 1, 2, 0)                         # [w, c, i, b]
    for q in range(2):
        xi[q, :MP, 1:65, :] = x0t[wm[q], cm]
        hw = 32 if q == 0 else 31                          # halo w value
        xi[q, 96:99, 1:65, :] = x0t[hw]
    x_init = xi.astype(F16)

    # noise' [18, 2, 96, 64, 16] in the psum/M order of each chunk
    npr = np.zeros((NUM_STEPS, 2, MP, H, BS), F32)
    eps = noise[:, rows].astype(np.float64)                # [18, 16, C, H, W]
    for s in range(NUM_STEPS):
        A, Bc, Bs, Cs, Ss = coeffs[s]
        n = Ss * eps[s] + Cs * mu + (Bs * np.asarray(b_net, np.float64))[None, :, None, None]
        nt = n.transpose(3, 1, 2, 0)                       # [w, c, i, b]
        for q in range(2):
            npr[s, q] = nt[wm[q], cm]
    noise_p = npr.astype(F16)

    if "wts" not in shared:
        shared["wts"] = _build_toeplitz(np.asarray(W_net, np.float64))
        # classifier weights, permuted to the staged feature order:
        # F = q*6144 + p*64 + i ; f_orig = c*4096 + i*64 + wmap_q(p)
        qv, pv, iv = np.meshgrid(np.arange(2), np.arange(MP), np.arange(64),
                                 indexing="ij")
        wv = np.where(qv == 0, (pv // 3 + 31) % 32, 32 + pv // 3)
        f_orig = ((pv % 3) * 4096 + iv * 64 + wv).reshape(-1)
        w2 = (0.5 * W_cls.astype(np.float64))[f_orig].astype(F16)
        w2 = w2.reshape(NW2G, W2GRP, 128, NUM_CLASSES)
        shared["w2"] = np.ascontiguousarray(w2.transpose(0, 2, 1, 3))
        bc2 = (np.asarray(b_cls, np.float64) +
               0.5 * W_cls.astype(np.float64).sum(0)).astype(F16)
        shared["bc2"] = bc2.reshape(1, NUM_CLASSES)
        pair = np.zeros((BS, BPC), F16)
        for j in range(BPC):
            pair[j, j] = 1.0
            pair[BPC + j, j] = 1.0
        shared["pair"] = pair
        shared["i96"] = np.eye(MP, dtype=F16)

    return {"x_init": x_init, "noise": noise_p, "wts": shared["wts"],
            "w2": shared["w2"], "bc2": shared["bc2"], "pair": shared["pair"],
            "i96": shared["i96"]}


# ---------------------------------------------------------------------------
_CACHE = {}


def _build_bass():
    import concourse.bacc as bacc
    import concourse.tile as tile
    import concourse.mybir as mybir

    nc = bacc.Bacc("TRN2", target_bir_lowering=False, debug=False)
    names = {}
    with tile.TileContext(nc) as tc:
        with tc.tile_pool(name="dram", bufs=1, space="DRAM") as dram, \
             tc.tile_pool(name="const", bufs=1) as const, \
             tc.tile_pool(name="noisep", bufs=4) as noisep, \
             tc.tile_pool(name="w2p", bufs=W2BUFS) as w2p, \
             tc.tile_pool(name="psamp", bufs=1, space="PSUM") as psamp, \
             tc.tile_pool(name="pcls", bufs=1, space="PSUM") as pcls:

            f16, f32 = mybir.dt.float16, mybir.dt.float32
            CopyF = mybir.ActivationFunctionType.Copy
            x_init_d = dram.tile([2, KP, HW_PAD, BS], f16, kind="ExternalInput")
            noise_d = dram.tile([NUM_STEPS, 2, MP, H, BS], f16, kind="ExternalInput")
            wts_d = dram.tile([KP, NUM_STEPS * 6, MP], f16, kind="ExternalInput")
            w2_d = dram.tile([NW2G, 128, W2GRP, NUM_CLASSES], f16,
                             kind="ExternalInput")
            bc2_d = dram.tile([1, NUM_CLASSES], f16, kind="ExternalInput")
            pair_d = dram.tile([BS, BPC], f16, kind="ExternalInput")
            i96_d = dram.tile([MP, MP], f16, kind="ExternalInput")
            out_d = dram.tile([BPC, NUM_CLASSES], f32, kind="ExternalOutput")
            stage2 = [dram.tile([MP * H, BS], f16, name=f"stage2_{qq}")
                      for qq in range(2)]
            names.update(x_init=x_init_d.name, noise=noise_d.name,
                         wts=wts_d.name, w2=w2_d.name, bc2=bc2_d.name,
                         pair=pair_d.name, i96=i96_d.name, out=out_d.name)

            # init loads: x first (gates w2 prefetch), then step-0 noise, wts
            x_sb = [[None, None], [None, None]]
            for q in range(2):
                for pp in range(2):
                    t = const.tile([KP, HW_PAD, BS], f16, tag=f"x{q}{pp}",
                                   name=f"x_sb{q}{pp}")
                    x_sb[q][pp] = t
            nztiles = {}

            def load_noise(s, engine):
                for q in range(2):
                    t = noisep.tile([MP, H, BS], f16, tag=f"nz{q}",
                                    name=f"nz{s}_{q}")
                    engine.dma_start(out=t, in_=noise_d[s, q])
                    nztiles[(s, q)] = t

            load_noise(0, nc.sync)
            nc.sync.dma_start(out=x_sb[0][0], in_=x_init_d[0])
            nc.sync.dma_start(out=x_sb[1][0], in_=x_init_d[1])
            nc.vector.memset(x_sb[0][1][:], 0.0)
            nc.vector.memset(x_sb[1][1][:], 0.0)
            WSPLIT = 4 * 6
            wts_a = const.tile([KP, WSPLIT, MP], f16)
            nc.sync.dma_start(out=wts_a, in_=wts_d[:, 0:WSPLIT, :])
            i96_sb = const.tile([MP, MP], f16)
            nc.sync.dma_start(out=i96_sb, in_=i96_d)
            for sq in range(1, 4):
                load_noise(sq, nc.sync)
            wts_b = const.tile([KP, NUM_STEPS * 6 - WSPLIT, MP], f16)
            nc.sync.dma_start(out=wts_b, in_=wts_d[:, WSPLIT:, :])

            def wts(s, dy, q):
                j = (s * 3 + dy) * 2 + q
                if j < WSPLIT:
                    return wts_a[:, j, :]
                return wts_b[:, j - WSPLIT, :]
            bc2_sb = const.tile([1, NUM_CLASSES], f16)
            nc.sync.dma_start(out=bc2_sb, in_=bc2_d)
            pair_sb = const.tile([BS, BPC], f16)
            nc.sync.dma_start(out=pair_sb, in_=pair_d)
            ones_sb = const.tile([1, BS], f16)
            nc.vector.memset(ones_sb[:], 1.0)
            w2tiles = []

            psum_u = [psamp.tile([MP, H, BPC], f32, tag=f"ps{u}",
                                 name=f"psum_u{u}") for u in range(4)]

            for s in range(NUM_STEPS):
                rd, wr = s % 2, (s + 1) % 2
                if 1 <= s and s + 3 < NUM_STEPS:
                    load_noise(s + 3, nc.sync)
                nz = [nztiles[(s, 0)], nztiles[(s, 1)]]
                order = [(1, 0), (1, 1), (0, 0), (0, 1)] if s % 2 == 0 else \
                        [(0, 0), (0, 1), (1, 0), (1, 1)]
                for q, bh in order:
                    ps = psum_u[2 * q + bh]
                    bsl = slice(BPC * bh, BPC * bh + BPC)
                    nc.tensor.matmul(
                        out=ps[:], lhsT=i96_sb[:], rhs=nz[q][0:MP, :, bsl],
                        start=True, stop=False)
                    for dy in range(3):
                        nc.tensor.matmul(
                            out=ps[:],
                            lhsT=wts(s, dy, q),
                            rhs=x_sb[q][rd][0:KP, dy:dy + H, bsl],
                            start=False, stop=(dy == 2))
                    nc.vector.tensor_copy(
                        out=x_sb[q][wr][0:MP, 1:H + 1, bsl], in_=ps[:])
                    if s < NUM_STEPS - 1:
                        nc.scalar.activation(
                            out=x_sb[1 - q][wr][96:99, 1:H + 1, bsl],
                            in_=ps[0:3, :, :], func=CopyF)
                if 2 <= s < 14:
                    # W2 prefetch, paced at one 1MB group per step: a tiny
                    # gate DMA writes a corner of the group's tile and reads
                    # this step's output, so the scheduler cannot start the
                    # real load before this step finishes (WAW on the tile)
                    w2t = w2p.tile([128, W2GRP, NUM_CLASSES], f16,
                                   tag="w2", name="w2t")
                    nc.gpsimd.dma_start(out=w2t[0:1, 0, 0:BS],
                                        in_=x_sb[0][wr][0:1, 1, :])
                    nc.gpsimd.dma_start(out=w2t, in_=w2_d[s - 2])
                    w2tiles.append(w2t)

            # ---- classifier (final state is in buffer 0) ----
            fin = NUM_STEPS % 2
            xT_sb = []
            for q in range(2):
                sview = stage2[q].rearrange("(p i) b -> p i b", p=MP)
                nc.sync.dma_start(out=sview, in_=x_sb[q][fin][0:MP, 1:H + 1, :])
                t = const.tile([128, NKC // 2, BS], f16, tag=f"xT{q}",
                               name=f"xT_sb{q}")
                nc.sync.dma_start(
                    out=t[:],
                    in_=stage2[q].rearrange("(ck fi) b -> fi ck b", fi=128))
                xT_sb.append(t)
            for g in range(12, NW2G):
                w2t = w2p.tile([128, W2GRP, NUM_CLASSES], f16, tag="w2",
                               name="w2t")
                nc.gpsimd.dma_start(out=w2t, in_=w2_d[g])
                w2tiles.append(w2t)
            psum_cls = pcls.tile([BS, 1024], f32, tag="cls")
            for kk in range(NKC):
                w2t = w2tiles[kk // W2GRP]
                j = kk % W2GRP
                xT = xT_sb[kk // (NKC // 2)][:, kk % (NKC // 2), :]
                nc.tensor.matmul(out=psum_cls[:, 0:512], lhsT=xT,
                                 rhs=w2t[:, j, 0:512], start=(kk == 0),
                                 stop=False)
                nc.tensor.matmul(out=psum_cls[:, 512:NUM_CLASSES], lhsT=xT,
                                 rhs=w2t[:, j, 512:NUM_CLASSES],
                                 start=(kk == 0), stop=False)
            nc.tensor.matmul(out=psum_cls[:, 0:512], lhsT=ones_sb[:],
                             rhs=bc2_sb[:, 0:512], start=False, stop=True)
            nc.tensor.matmul(out=psum_cls[:, 512:NUM_CLASSES], lhsT=ones_sb[:],
                             rhs=bc2_sb[:, 512:NUM_CLASSES], start=False,
                             stop=True)

            e_sb = const.tile([BS, NUM_CLASSES], f16)
            nc.scalar.activation(out=e_sb[:], in_=psum_cls[:, 0:NUM_CLASSES],
                                 func=mybir.ActivationFunctionType.Exp)
            psum_lse = pcls.tile([BPC, 1024], f32, tag="lse")
            nc.tensor.matmul(out=psum_lse[:, 0:512], lhsT=pair_sb[:],
                             rhs=e_sb[:, 0:512], start=True, stop=True)
            nc.tensor.matmul(out=psum_lse[:, 512:NUM_CLASSES], lhsT=pair_sb[:],
                             rhs=e_sb[:, 512:NUM_CLASSES], start=True, stop=True)
            lse_sb = const.tile([BPC, NUM_CLASSES], f32)
            nc.scalar.activation(out=lse_sb[:], in_=psum_lse[:, 0:NUM_CLASSES],
                                 func=mybir.ActivationFunctionType.Ln, scale=0.5)
            nc.sync.dma_start(out=out_d, in_=lse_sb)

    nc.compile()
    return nc, names


def get_built():
    if "nc" not in _CACHE:
        _CACHE["nc"], _CACHE["names"] = _build_bass()
    return _CACHE["nc"], _CACHE["names"]


def make_in_maps(x, latents, noise, W_net, b_net, W_cls, b_cls):
    nc, names = get_built()
    shared = {}
    in_maps = []
    for core in range(NCORES):
        arrs = _host_prep(core, x, latents, noise, W_net, b_net, W_cls,
                          b_cls, shared)
        in_maps.append({names[k]: arrs[k] for k in
                        ("x_init", "noise", "wts", "w2", "bc2", "pair", "i96")})
    return in_maps


def kernel(x, latents, noise, W_net, b_net, W_cls, b_cls):
    from concourse import bass_utils
    nc, names = get_built()
    in_maps = make_in_maps(x, latents, noise, W_net, b_net, W_cls, b_cls)
    trace = bool(int(os.environ.get("CEC_TRACE", "0")))
    res = bass_utils.run_bass_kernel_spmd(
        nc, in_maps, core_ids=list(range(NCORES)), trace=trace)
    _CACHE["last_results"] = res
    out = np.zeros((B, NUM_CLASSES), np.float32)
    for core in range(NCORES):
        out[BPC * core:BPC * core + BPC] = res.results[core][names["out"]]
    return out

